# revision 19
# baseline (speedup 1.0000x reference)
"""MoE top-2 routing kernel for Trainium2, 8-core data-parallel.

Problem: x [524288, 128] f32; gate Linear(128->8); 8 experts Linear(128->128).
  g = softmax(x @ gate_W.T + gate_b); top-2 mask; out = sum_e (g*mask)_e * (x @ W_e.T) + g @ b

Per core (65536 tokens): groups of 8 tiles x 128 tokens.
  pass 1 (per tile): DMA x, PE transpose -> xT (f32r), gate matmul -> group logits psum
  pass 2 (per group): batched softmax + top-2 mask + gT transpose (bf16)
  pass 3 (per tile): expert matmuls (f32r, N=512 x2) -> yall psum; bias matmul (bf16);
    weighted reduce: one broadcast tensor_tensor mult (bf16 out) + bf16 add tree + bias add.
"""

import sys

if "/opt/trn_rl_repo" not in sys.path:
    sys.path.insert(0, "/opt/trn_rl_repo")

from contextlib import ExitStack

import ml_dtypes
import numpy as np

import concourse.bass as bass
import concourse.tile as tile
from concourse import bacc
from concourse import mybir

F32 = mybir.dt.float32
F32R = mybir.dt.float32r
BF16 = mybir.dt.bfloat16
AF = mybir.ActivationFunctionType
OP = mybir.AluOpType
AX = mybir.AxisListType

N_TOKENS = 524288
D = 128
E = 8
N_CORES = 8
P = 128
G = 16  # tiles per group


def _bcast_inner(ap, n_outer, rep_len):
    """View [P, n_outer] as [P, n_outer, rep_len] with inner dim broadcast (step 0)."""
    return bass.AP(
        tensor=ap.tensor,
        offset=ap.offset,
        ap=[ap.ap[0], [ap.ap[-1][0], n_outer], [0, rep_len]],
    )


def build_nc(shard_tokens: int, inner_tiles: int = G) -> bass.Bass:
    ntiles = shard_tokens // P
    assert ntiles % inner_tiles == 0
    outer = ntiles // inner_tiles
    gi = inner_tiles

    nc = bacc.Bacc()
    x = nc.dram_tensor("x", [shard_tokens, D], F32R, kind="ExternalInput")
    # wcat[d, e*128+f] = W[e, f, d]; wcat[d, 1024+e] = gate_W[e, d]
    wcat = nc.dram_tensor("wcat", [D, E * D + E], F32R, kind="ExternalInput")
    gb8 = nc.dram_tensor("gb8", [P, gi * E], F32, kind="ExternalInput")
    b_bf = nc.dram_tensor("b_bf", [E, D], BF16, kind="ExternalInput")
    b4 = nc.dram_tensor("b4", [P, D], BF16, kind="ExternalInput")
    ident_f = nc.dram_tensor("ident_f", [P, P], F32R, kind="ExternalInput")
    ident_bf = nc.dram_tensor("ident_bf", [P, P], BF16, kind="ExternalInput")
    out = nc.dram_tensor("out", [shard_tokens, D], F32, kind="ExternalOutput")

    x_v = x.rearrange("(n a p) d -> n p a d", p=P, a=gi)
    out_v = out.rearrange("(n a p) d -> n p a d", p=P, a=gi)

    with ExitStack() as ctx:
        tc = ctx.enter_context(tile.TileContext(nc))
        consts = ctx.enter_context(tc.tile_pool(name="consts", bufs=1))
        io_pool = ctx.enter_context(tc.tile_pool(name="io", bufs=2))
        xt_pool = ctx.enter_context(tc.tile_pool(name="xts", bufs=2))
        work = ctx.enter_context(tc.tile_pool(name="work", bufs=2))
        gates = ctx.enter_context(tc.tile_pool(name="gates", bufs=2))
        psum_y = ctx.enter_context(tc.tile_pool(name="psum_y", bufs=2, space="PSUM"))
        psum_t = ctx.enter_context(tc.tile_pool(name="psum_t", bufs=2, space="PSUM"))
        psum_g = ctx.enter_context(tc.tile_pool(name="psum_g", bufs=2, space="PSUM"))

        # ---- constants (one-time) ----
        wcat_sb = consts.tile([D, E * D + E], F32R)
        nc.sync.dma_start(out=wcat_sb, in_=wcat[:, :])
        gb_sb = consts.tile([P, gi * E], F32)
        nc.sync.dma_start(out=gb_sb, in_=gb8[:, :])
        b_sb = consts.tile([E, D], BF16)
        nc.sync.dma_start(out=b_sb, in_=b_bf[:, :])
        b4_sb = consts.tile([P, D], BF16)
        nc.sync.dma_start(out=b4_sb, in_=b4[:, :])
        ident_r = consts.tile([P, P], F32R)
        nc.sync.dma_start(out=ident_r, in_=ident_f[:, :])
        ident_b = consts.tile([P, P], BF16)
        nc.sync.dma_start(out=ident_b, in_=ident_bf[:, :])

        wmov = wcat_sb[:, 0 : E * D]
        wgate = wcat_sb[:, E * D : E * D + E]

        def body(base):
            x_in = io_pool.tile([P, gi, D], F32R, tag="x_in")
            nc.sync.dma_start(out=x_in, in_=x_v[base])
            out_sb = io_pool.tile([P, gi, D], F32, tag="out_sb")

            # group psum: logits fp32 in [:, 0:gi*E]; gT bf16 staging at bytes 512+
            lgp = psum_g.tile([P, 512], F32, tag="lgp")
            xts = xt_pool.tile([P, gi, D], F32R, tag="xts")

            # ---- pass 1: transpose + gate ----
            for j in range(gi):
                tp = psum_t.tile([P, D], F32, tag="tp")
                nc.tensor.transpose(tp.bitcast(F32R), x_in[:, j, :], ident_r)
                nc.scalar.copy(xts[:, j, :], tp)
                nc.tensor.matmul(
                    lgp[:, j * E : (j + 1) * E],
                    xts[:, j, :].bitcast(F32),
                    wgate.bitcast(F32),
                    start=True,
                    stop=True,
                )

            # ---- pass 2: batched softmax/top2 over [P, gi*E] ----
            ge = gi * E
            lg = gates.tile([P, ge], F32, tag="lg")
            nc.vector.tensor_tensor(out=lg, in0=lgp[:, 0:ge], in1=gb_sb, op=OP.add)
            lg3 = lg.rearrange("p (a e) -> p a e", e=E)
            eg = gates.tile([P, ge], F32, tag="eg")
            nc.scalar.activation(eg, lg, AF.Exp)
            eg3 = eg.rearrange("p (a e) -> p a e", e=E)
            m1 = gates.tile([P, gi], F32, tag="m1")
            nc.vector.tensor_reduce(out=m1, in_=lg3, axis=AX.X, op=OP.max)
            s8 = gates.tile([P, gi], F32, tag="s8")
            nc.vector.tensor_reduce(out=s8, in_=eg3, axis=AX.X, op=OP.add)
            r8 = gates.tile([P, gi], F32, tag="r8")
            nc.vector.reciprocal(r8, s8)
            eq1 = gates.tile([P, ge], F32, tag="eq1")
            nc.vector.tensor_tensor(
                out=eq1, in0=lg, in1=_bcast_inner(m1, gi, E), op=OP.is_equal
            )
            msk1 = gates.tile([P, ge], F32, tag="msk1")
            nc.vector.scalar_tensor_tensor(
                out=msk1, in0=eq1, scalar=-1e30, in1=lg, op0=OP.mult, op1=OP.add
            )
            msk13 = msk1.rearrange("p (a e) -> p a e", e=E)
            m2 = gates.tile([P, gi], F32, tag="m2")
            nc.vector.tensor_reduce(out=m2, in_=msk13, axis=AX.X, op=OP.max)
            mk = gates.tile([P, ge], F32, tag="mk")
            nc.vector.tensor_tensor(
                out=mk, in0=lg, in1=_bcast_inner(m2, gi, E), op=OP.is_ge
            )
            gu = gates.tile([P, ge], F32, tag="gu")
            nc.vector.tensor_tensor(
                out=gu, in0=eg, in1=_bcast_inner(r8, gi, E), op=OP.mult
            )
            gh = gates.tile([P, ge], F32, tag="gh")
            nc.vector.tensor_tensor(out=gh, in0=gu, in1=mk, op=OP.mult)
            # gT for the bias matmuls: gu copied (bf16) into padded slots so each
            # tile's 8 gates land at partition offset 32*(j%4) after transposing.
            nh = gi // 4
            gu_pad = gates.tile([P, nh, 4, 32], BF16, tag="gu_pad")
            nc.vector.memset(gu_pad, 0.0)
            nc.vector.tensor_copy(
                out=gu_pad[:, :, :, 0:E],
                in_=gu.rearrange("p (h q e) -> p h q e", q=4, e=E),
            )
            gt2 = gates.tile([P, nh, P], BF16, tag="gt2")
            goff = 2 * ((ge + 127) // 128) * 64  # fp32 cols used by logits, 64-aligned
            for h in range(nh):
                gt_ps = lgp[:, goff + 64 * h : goff + 64 * (h + 1)].bitcast(BF16)[:, 0:P]
                nc.tensor.transpose(
                    gt_ps, gu_pad[:, h, :, :].rearrange("p q e -> p (q e)"), ident_b
                )
                nc.scalar.copy(gt2[:, h, :], gt_ps)

            # ---- pass 3: experts + weighted reduce ----
            for j in range(gi):
                yp = psum_y.tile([P, E * D], F32, tag="yall")
                nc.tensor.matmul(
                    yp[:, 0:512], xts[:, j, :], wmov[:, 0:512], start=True, stop=True
                )
                nc.tensor.matmul(
                    yp[:, 512:1024],
                    xts[:, j, :],
                    wmov[:, 512:1024],
                    start=True,
                    stop=True,
                )
                bp = psum_t.tile([P, D], F32, tag="tp")
                h, q = j // 4, j % 4
                nc.tensor.matmul(
                    bp,
                    gt2[32 * q : 32 * q + E, h, :],
                    b4_sb[32 * q : 32 * q + E, :],
                    start=True,
                    stop=True,
                    tile_position=(32 * q, 0),
                )

                # mult-pass (e-outer layout): sc[p, e, f] = yall[p, e, f] * gh[p, j, e]
                # experts 0..5 on DVE (one broadcast op), 6..7 on ACT scaled copies
                sc = work.tile([P, E, D], BF16, tag="sc")
                yp3 = yp.rearrange("p (e f) -> p e f", f=D)
                ghj = gh[:, j * E : (j + 1) * E]
                ghb = bass.AP(
                    tensor=ghj.tensor,
                    offset=ghj.offset,
                    ap=[ghj.ap[0], [1, 6], [0, D]],
                )
                nc.vector.tensor_tensor(
                    out=sc[:, 0:6, :], in0=yp3[:, 0:6, :], in1=ghb, op=OP.mult
                )
                for e in (6, 7):
                    nc.scalar.activation(
                        sc[:, e, :],
                        yp3[:, e, :],
                        AF.Copy,
                        scale=ghj[:, e : e + 1],
                    )
                # bf16 add tree over e: level 1 on gpsimd, 2-3 on DVE
                sc4 = work.tile([P, 4, D], BF16, tag="sc4")
                nc.gpsimd.tensor_tensor(
                    out=sc4, in0=sc[:, 0:4, :], in1=sc[:, 4:8, :], op=OP.add
                )
                sc2 = work.tile([P, 2, D], BF16, tag="sc2")
                nc.vector.tensor_tensor(
                    out=sc2, in0=sc4[:, 0:2, :], in1=sc4[:, 2:4, :], op=OP.add
                )
                s1 = work.tile([P, D], BF16, tag="s1")
                nc.vector.tensor_tensor(
                    out=s1, in0=sc2[:, 0, :], in1=sc2[:, 1, :], op=OP.add
                )
                # final: out = s1 + bias_psum
                nc.vector.tensor_tensor(out=out_sb[:, j, :], in0=bp, in1=s1, op=OP.add)

            nc.sync.dma_start(out=out_v[base], in_=out_sb)

        if outer == 1:
            body(0)
        else:
            with tc.For_i(0, outer, 1) as it:
                body(it)

    nc.compile()
    return nc


def _prep_consts(gate_W, gate_b, W, b):
    wcat = np.concatenate(
        [W.transpose(2, 0, 1).reshape(D, E * D), gate_W.T], axis=1
    ).astype(np.float32)
    gb8 = np.tile(gate_b.astype(np.float32), (P, G))
    b_bf = b.astype(ml_dtypes.bfloat16)
    ident_f = np.eye(P, dtype=np.float32)
    ident_bf = np.eye(P, dtype=ml_dtypes.bfloat16)
    b4 = np.zeros((P, D), dtype=ml_dtypes.bfloat16)
    for k in range(4):
        b4[32 * k : 32 * k + E] = b.astype(ml_dtypes.bfloat16)
    return wcat, gb8, b_bf, b4, ident_f, ident_bf


_NC_CACHE = {}


def _get_nc(shard_tokens):
    if shard_tokens not in _NC_CACHE:
        _NC_CACHE[shard_tokens] = build_nc(shard_tokens)
    return _NC_CACHE[shard_tokens]


def kernel(**inputs) -> np.ndarray:
    x = np.ascontiguousarray(np.asarray(inputs["x"], dtype=np.float32))
    gate_W = np.asarray(inputs["gate_W"], dtype=np.float32)
    gate_b = np.asarray(inputs["gate_b"], dtype=np.float32)
    W = np.asarray(inputs["W"], dtype=np.float32)
    b = np.asarray(inputs["b"], dtype=np.float32)

    n = x.shape[0]
    shard = n // N_CORES
    wcat, gb8, b_bf, b4, ident_f, ident_bf = _prep_consts(gate_W, gate_b, W, b)

    nc = _get_nc(shard)
    in_maps = [
        {
            "x": x[c * shard : (c + 1) * shard],
            "wcat": wcat,
            "gb8": gb8,
            "b_bf": b_bf,
            "b4": b4,
            "ident_f": ident_f,
            "ident_bf": ident_bf,
        }
        for c in range(N_CORES)
    ]
    from concourse.bass_utils import run_bass_kernel_spmd

    res = run_bass_kernel_spmd(nc, in_maps, core_ids=list(range(N_CORES)))
    out = np.concatenate([res.results[c]["out"] for c in range(N_CORES)], axis=0)
    return out.astype(np.float32)
